# revision 9
# baseline (speedup 1.0000x reference)
"""BatchHardContrastiveLoss Trainium2 kernel (8-core SPMD).

Math: dist^2[i,j] = sq[i] + sq[j] - 2*x_i.x_j.  Per anchor i we need
  hardest_pos[i] = max over positives j of dist[i,j]
  hardest_neg[i] = min over negatives j of dist[i,j]
Monotonicity of sqrt lets us mine in the d2 domain.  On device we compute
  t'[i,j] = s*(sq[j] - 2*g[i,j])          (PE, bf16 matmuls, fp32 PSUM)
and fuse mask-application with the row reduction in a single DVE op
(tensor_tensor_reduce):
  r_pos[i] = max_j (t'[i,j] - b_pos[i,j]),  b_pos = 0 if positive else FILL
  r_neg[i] = min_j (t'[i,j] + b_neg[i,j]),  b_neg = 0 if negative else FILL
FILL separates masked from unmasked values (s is chosen so the total spread
of t' is < FILL), and doubles as the empty-row detector.  Host adds s*sq[i],
takes sqrt, and applies the margins / AvgNonZero reduction (O(N) work).
"""

import numpy as np
import ml_dtypes

import concourse.bass as bass
import concourse.mybir as mybir
import concourse.tile as tile
from concourse import bacc
from concourse import dve_ops as _dvo
from concourse.bass_utils import run_bass_kernel_spmd
from concourse.dve_spec import C0, Spec, Src0, Src1, _has_src1, lower, maxx, minn
from concourse.dve_table_gen import dve_ver_for
from concourse.dve_uop import DveOpSpec


def _register_dve_op(name, spec):
    """Register a custom DVE op at runtime (idempotent). The uop sha is
    computed here and pinned, so DveOp.compile's drift check always passes."""
    for op in _dvo.OPS:
        if op.name == name:
            return op
    row = _dvo._CUSTOM_DVE_ROW_BASE + len(_dvo.OPS)
    assert row < 0x20, "custom-DVE sub-opcode rows exhausted"
    _dvo._SUB_OPCODE_FOR_NAME[name] = row
    ver = dve_ver_for("TRN2")
    uops = lower(spec, ver=ver)
    sha = DveOpSpec(name=name, opcode=row, uops=uops, rd1_en=_has_src1(spec)).sha(ver)
    op = _dvo.DveOp(name, spec, subdim=False, uops_sha={ver: sha})
    _dvo.OPS.append(op)
    _dvo.CUSTOM_DVE_SPECS[name] = spec
    return op


# accum_out = max(s0, max_j(in0 - in1)) / min(s0, min_j(in0 + in1))
SUB_MAX = _register_dve_op(
    "ANT_SUB_MAX_REDUCE", Spec(body=Src0 - Src1, accum=maxx, accum_init=C0)
)
ADD_MIN = _register_dve_op(
    "ANT_ADD_MIN_REDUCE", Spec(body=Src0 + Src1, accum=minn, accum_init=C0)
)

N_CORES = 8
FILL = 240.0
POS_MARGIN = 0.2
NEG_MARGIN = 0.2

BF16 = mybir.dt.bfloat16
F32 = mybir.dt.float32
U8 = mybir.dt.uint8


def build_nc(R, N, D, jch=2048, seg=512, repeat=1):
    """One-core program; run SPMD on all cores with per-core inputs.

    R: anchor rows owned by this core, N: total columns, D: embed dim.
    repeat>1 wraps the whole body in a device-side loop (for timing by
    differencing; max/min accumulation is idempotent across repeats).
    """
    assert R % 128 == 0 and N % jch == 0 and jch % seg == 0 and D % 128 == 0
    n_ib = R // 128
    n_jc = N // jch
    n_seg = jch // seg
    n_k = D // 128

    nc = bacc.Bacc(None, target_bir_lowering=False)
    lhsT_d = nc.dram_tensor("lhsT", [D, R], BF16, kind="ExternalInput")
    rhs_d = nc.dram_tensor("rhs", [D, N], BF16, kind="ExternalInput")
    sqr_d = nc.dram_tensor("sqrow", [2, N], BF16, kind="ExternalInput")
    ones_d = nc.dram_tensor("ones2", [2, 128], BF16, kind="ExternalInput")
    bpos_d = nc.dram_tensor("bpos", [R, N], U8, kind="ExternalInput")
    bneg_d = nc.dram_tensor("bneg", [R, N], U8, kind="ExternalInput")
    rpos_d = nc.dram_tensor("rpos", [128, n_ib], F32, kind="ExternalOutput")
    rneg_d = nc.dram_tensor("rneg", [128, n_ib], F32, kind="ExternalOutput")

    with tile.TileContext(nc) as tc:
        with (
            tc.tile_pool(name="const", bufs=1) as cpool,
            tc.tile_pool(name="masks", bufs=2) as mpool,
            tc.tile_pool(name="psum", bufs=2, space="PSUM") as ppool,
            tc.tile_pool(name="acc", bufs=1) as apool,
        ):
            rhs_sb = []
            lhs_sb = []
            for k in range(n_k):
                rk = cpool.tile([128, N], BF16, tag=f"rhs{k}", name=f"rhs_sb{k}")
                lk = cpool.tile([128, R], BF16, tag=f"lhs{k}", name=f"lhs_sb{k}")
                nc.sync.dma_start(rk[:], rhs_d[k * 128 : (k + 1) * 128, :])
                nc.sync.dma_start(lk[:], lhsT_d[k * 128 : (k + 1) * 128, :])
                rhs_sb.append(rk)
                lhs_sb.append(lk)
            sqr_sb = cpool.tile([2, N], BF16, tag="sqr", name="sqr_sb")
            ones_sb = cpool.tile([2, 128], BF16, tag="ones", name="ones_sb")
            nc.sync.dma_start(sqr_sb[:], sqr_d[:])
            nc.sync.dma_start(ones_sb[:], ones_d[:])

            accp = apool.tile([128, n_ib], F32, tag="accp", name="accp")
            accn = apool.tile([128, n_ib], F32, tag="accn", name="accn")
            dum0 = apool.tile([128, 1], F32, tag="dum0", name="dum0")
            dum1 = apool.tile([128, 1], F32, tag="dum1", name="dum1")

            def trace_body():
              for ib in range(n_ib):
                bp = mpool.tile([128, N], U8, tag="bp", name=f"bp{ib}")
                bn = mpool.tile([128, N], U8, tag="bn", name=f"bn{ib}")
                nc.sync.dma_start(bp[:], bpos_d[ib * 128 : (ib + 1) * 128, :])
                nc.sync.dma_start(bn[:], bneg_d[ib * 128 : (ib + 1) * 128, :])
                ibsl = slice(ib * 128, (ib + 1) * 128)
                for jc in range(n_jc):
                    pt = ppool.tile([128, jch], F32, tag="pt", name=f"pt{ib}_{jc}")
                    for sg in range(n_seg):
                        j0 = jc * jch + sg * seg
                        osl = slice(sg * seg, (sg + 1) * seg)
                        nc.tensor.matmul(
                            pt[:, osl], lhs_sb[0][:, ibsl], rhs_sb[0][:, j0 : j0 + seg],
                            start=True, stop=False,
                        )
                        for k in range(1, n_k):
                            nc.tensor.matmul(
                                pt[:, osl], lhs_sb[k][:, ibsl],
                                rhs_sb[k][:, j0 : j0 + seg],
                                start=False, stop=False,
                            )
                        nc.tensor.matmul(
                            pt[:, osl], ones_sb[:], sqr_sb[:, j0 : j0 + seg],
                            start=False, stop=True,
                        )
                    jsl = slice(jc * jch, (jc + 1) * jch)
                    nc.vector._custom_dve(
                        SUB_MAX,
                        out=dum0.broadcast_to((128, jch)),
                        in0=pt[:],
                        in1=bp[:, jsl],
                        s0=(-1e30 if jc == 0 else accp[:, ib : ib + 1]),
                        s1=0.0,
                        accum_out=accp[:, ib : ib + 1],
                    )
                    nc.vector._custom_dve(
                        ADD_MIN,
                        out=dum1.broadcast_to((128, jch)),
                        in0=pt[:],
                        in1=bn[:, jsl],
                        s0=(1e30 if jc == 0 else accn[:, ib : ib + 1]),
                        s1=0.0,
                        accum_out=accn[:, ib : ib + 1],
                    )

            if repeat == 1:
                trace_body()
            else:
                with tc.For_i(0, repeat, 1):
                    trace_body()
            nc.sync.dma_start(rpos_d[:], accp[:])
            nc.sync.dma_start(rneg_d[:], accn[:])
    nc.compile()
    return nc


def _avg_nonzero(losses):
    nz = np.count_nonzero(losses > 0)
    return float(np.sum(losses) / nz) if nz > 0 else 0.0


def _prep_inputs(embeddings, positives_mask, negatives_mask, n_cores):
    x = np.asarray(embeddings, dtype=np.float32)
    pos = np.asarray(positives_mask).astype(bool)
    neg = np.asarray(negatives_mask).astype(bool)
    n, d = x.shape
    r = n // n_cores

    sq = np.sum(x.astype(np.float64) ** 2, axis=1)
    sq_max = float(sq.max())
    sq_min = float(sq.min())
    # scale so the full spread of t' = s*(sq_j - 2g) fits inside FILL with
    # margin; |g| <= sq_max by Cauchy-Schwarz. Power of two => exact scaling.
    s = 0.125
    while s * (5.0 * sq_max - sq_min) > FILL - 32.0 and s > 2.0**-40:
        s *= 0.5

    rhs = (-2.0 * s * x.T).astype(ml_dtypes.bfloat16)  # [D, N]
    sqs = (s * sq).astype(np.float32)
    hi = sqs.astype(ml_dtypes.bfloat16)
    lo = (sqs - hi.astype(np.float32)).astype(ml_dtypes.bfloat16)
    sqrow = np.stack([hi, lo], axis=0)  # [2, N]
    ones2 = np.ones((2, 128), dtype=ml_dtypes.bfloat16)

    in_maps = []
    for c in range(n_cores):
        rows = slice(c * r, (c + 1) * r)
        in_maps.append(
            {
                "lhsT": np.ascontiguousarray(x[rows].T).astype(ml_dtypes.bfloat16),
                "rhs": rhs,
                "sqrow": sqrow,
                "ones2": ones2,
                "bpos": np.where(pos[rows], 0, int(FILL)).astype(np.uint8),
                "bneg": np.where(neg[rows], 0, int(FILL)).astype(np.uint8),
            }
        )
    aux = {"sq": sq, "s": s, "sq_max": sq_max, "sq_min": sq_min, "n": n, "r": r}
    return in_maps, aux


def _decode(results, aux, n_cores):
    sq, s = aux["sq"], aux["s"]
    n, r = aux["n"], aux["r"]
    n_ib = r // 128
    lo_t = s * (aux["sq_min"] - 2.0 * aux["sq_max"])
    hi_t = 3.0 * s * aux["sq_max"]
    th_pos = (lo_t + hi_t - FILL) / 2.0
    th_neg = (lo_t + hi_t + FILL) / 2.0

    r_pos = np.empty(n, dtype=np.float64)
    r_neg = np.empty(n, dtype=np.float64)
    for c in range(n_cores):
        rp = np.asarray(results[c]["rpos"], dtype=np.float64)  # [128, n_ib]
        rn = np.asarray(results[c]["rneg"], dtype=np.float64)
        base = c * r
        for ib in range(n_ib):
            r_pos[base + ib * 128 : base + (ib + 1) * 128] = rp[:, ib]
            r_neg[base + ib * 128 : base + (ib + 1) * 128] = rn[:, ib]

    has_pos = r_pos > th_pos
    has_neg = r_neg < th_neg
    valid = has_pos & has_neg

    d2_pos = r_pos / s + sq
    d2_neg = r_neg / s + sq
    hardest_pos = np.sqrt(np.maximum(d2_pos, 1e-12))
    hardest_neg = np.sqrt(np.maximum(d2_neg, 1e-12))
    pos_loss = np.where(valid, np.maximum(hardest_pos - POS_MARGIN, 0.0), 0.0)
    neg_loss = np.where(valid, np.maximum(NEG_MARGIN - hardest_neg, 0.0), 0.0)
    return np.float32(_avg_nonzero(pos_loss) + _avg_nonzero(neg_loss))


_NC_CACHE = {}


def _kernel_impl(embeddings, positives_mask, negatives_mask, trace=False):
    x = np.asarray(embeddings)
    n, d = x.shape
    in_maps, aux = _prep_inputs(embeddings, positives_mask, negatives_mask, N_CORES)
    key = (n // N_CORES, n, d)
    if key not in _NC_CACHE:
        _NC_CACHE[key] = build_nc(*key)
    nc = _NC_CACHE[key]
    out = run_bass_kernel_spmd(nc, in_maps, list(range(N_CORES)), trace=trace)
    result = _decode(out.results, aux, N_CORES)
    return result, out


def kernel(embeddings, positives_mask, negatives_mask):
    result, _ = _kernel_impl(embeddings, positives_mask, negatives_mask)
    return result


# revision 15
# speedup vs baseline: 6.7394x; 6.7394x over previous
"""BatchHardContrastiveLoss Trainium2 kernel (8-core SPMD).

Math: dist^2[i,j] = sq[i] + sq[j] - 2*x_i.x_j.  Per anchor i we need
  hardest_pos[i] = max over positives j of dist[i,j]
  hardest_neg[i] = min over negatives j of dist[i,j]
Monotonicity of sqrt lets us mine in the d2 domain.  On device we compute
  t'[i,j] = s*(sq[j] - 2*g[i,j])          (PE, bf16 matmuls, fp32 PSUM)
and fuse mask-application with the row reduction in a single DVE op
(tensor_tensor_reduce):
  r_pos[i] = max_j (t'[i,j] - b_pos[i,j]),  b_pos = 0 if positive else FILL
  r_neg[i] = min_j (t'[i,j] + b_neg[i,j]),  b_neg = 0 if negative else FILL
FILL separates masked from unmasked values (s is chosen so the total spread
of t' is < FILL), and doubles as the empty-row detector.  Host adds s*sq[i],
takes sqrt, and applies the margins / AvgNonZero reduction (O(N) work).
"""

import numpy as np
import ml_dtypes

import concourse.bass as bass
import concourse.mybir as mybir
import concourse.tile as tile
from concourse import bacc
from concourse import dve_ops as _dvo
from concourse.bass_utils import run_bass_kernel_spmd
from concourse.dve_spec import C0, Spec, Src0, Src1, _has_src1, lower, maxx, minn
from concourse.dve_table_gen import dve_ver_for
from concourse.dve_uop import DveOpSpec


def _register_dve_op(name, spec):
    """Register a custom DVE op at runtime (idempotent). The uop sha is
    computed here and pinned, so DveOp.compile's drift check always passes."""
    for op in _dvo.OPS:
        if op.name == name:
            return op
    row = _dvo._CUSTOM_DVE_ROW_BASE + len(_dvo.OPS)
    assert row < 0x20, "custom-DVE sub-opcode rows exhausted"
    _dvo._SUB_OPCODE_FOR_NAME[name] = row
    ver = dve_ver_for("TRN2")
    uops = lower(spec, ver=ver)
    sha = DveOpSpec(name=name, opcode=row, uops=uops, rd1_en=_has_src1(spec)).sha(ver)
    op = _dvo.DveOp(name, spec, subdim=False, uops_sha={ver: sha})
    _dvo.OPS.append(op)
    _dvo.CUSTOM_DVE_SPECS[name] = spec
    return op


# accum_out = max(s0, max_j(in0 - in1)) / min(s0, min_j(in0 + in1))
SUB_MAX = _register_dve_op(
    "ANT_SUB_MAX_REDUCE", Spec(body=Src0 - Src1, accum=maxx, accum_init=C0)
)
ADD_MIN = _register_dve_op(
    "ANT_ADD_MIN_REDUCE", Spec(body=Src0 + Src1, accum=minn, accum_init=C0)
)

# Combined-mask ops: one u8 plane c = 128*inv_pos + 64*inv_neg (in {0,64,128,192}).
# pos-inactive <=> c >= 128;  neg-inactive <=> ((c - 128*(c>=128)) * 2) >= 128.
# C1 = fill, C2 = 128.0, C0 = running accumulator seed.
from concourse.dve_spec import C1, C2  # noqa: E402

_pos_pred = Src0 - (Src1 >= C2) * C1
POS_MAX2 = _register_dve_op(
    "ANT_CMASK_POS_MAX", Spec(body=_pos_pred, accum=maxx, accum_init=C0)
)
_b = Src1 >= C2
_r = Src1 - _b * C2
_neg_pred = Src0 + ((_r + _r) >= C2) * C1
NEG_MIN2 = _register_dve_op(
    "ANT_CMASK_NEG_MIN", Spec(body=_neg_pred, accum=minn, accum_init=C0)
)

N_CORES = 8
FILL = 240.0
POS_MARGIN = 0.2
NEG_MARGIN = 0.2

BF16 = mybir.dt.bfloat16
F32 = mybir.dt.float32
U8 = mybir.dt.uint8


def build_nc(R, N, D, jch=2048, seg=512, repeat=1):
    """One-core program; run SPMD on all cores with per-core inputs.

    R: anchor rows owned by this core, N: total columns, D: embed dim.
    repeat>1 wraps the whole body in a device-side loop (for timing by
    differencing; max/min accumulation is idempotent across repeats).
    """
    assert R % 128 == 0 and N % jch == 0 and jch % seg == 0 and D % 128 == 0
    n_ib = R // 128
    n_jc = N // jch
    n_seg = jch // seg
    n_k = D // 128

    nc = bacc.Bacc(None, target_bir_lowering=False)
    lhsT_d = nc.dram_tensor("lhsT", [D, R], BF16, kind="ExternalInput")
    rhs_d = nc.dram_tensor("rhs", [D, N], BF16, kind="ExternalInput")
    sqr_d = nc.dram_tensor("sqrow", [2, N], BF16, kind="ExternalInput")
    ones_d = nc.dram_tensor("ones2", [2, 128], BF16, kind="ExternalInput")
    bmask_d = nc.dram_tensor("bmask", [R, N], U8, kind="ExternalInput")
    rpos_d = nc.dram_tensor("rpos", [128, n_ib], F32, kind="ExternalOutput")
    rneg_d = nc.dram_tensor("rneg", [128, n_ib], F32, kind="ExternalOutput")

    with tile.TileContext(nc) as tc:
        with (
            tc.tile_pool(name="const", bufs=1) as cpool,
            tc.tile_pool(name="masks", bufs=1) as mpool,
            tc.tile_pool(name="psum", bufs=2, space="PSUM") as ppool,
            tc.tile_pool(name="acc", bufs=1) as apool,
        ):
            rhs_sb = []
            lhs_sb = []
            for k in range(n_k):
                rk = cpool.tile([128, N], BF16, tag=f"rhs{k}", name=f"rhs_sb{k}")
                lk = cpool.tile([128, R], BF16, tag=f"lhs{k}", name=f"lhs_sb{k}")
                nc.sync.dma_start(rk[:], rhs_d[k * 128 : (k + 1) * 128, :])
                nc.sync.dma_start(lk[:], lhsT_d[k * 128 : (k + 1) * 128, :])
                rhs_sb.append(rk)
                lhs_sb.append(lk)
            sqr_sb = cpool.tile([2, N], BF16, tag="sqr", name="sqr_sb")
            ones_sb = cpool.tile([2, 128], BF16, tag="ones", name="ones_sb")
            nc.sync.dma_start(sqr_sb[:], sqr_d[:])
            nc.sync.dma_start(ones_sb[:], ones_d[:])

            accp = apool.tile([128, n_ib], F32, tag="accp", name="accp")
            accn = apool.tile([128, n_ib], F32, tag="accn", name="accn")
            dum0 = apool.tile([128, 1], F32, tag="dum0", name="dum0")
            dum1 = apool.tile([128, 1], F32, tag="dum1", name="dum1")

            def trace_body():
              bms = []
              for ib in range(n_ib):
                bm = mpool.tile([128, N], U8, tag=f"bm{ib}", name=f"bm{ib}")
                nc.sync.dma_start(bm[:], bmask_d[ib * 128 : (ib + 1) * 128, :])
                bms.append(bm)
              for ib in range(n_ib):
                bm = bms[ib]
                ibsl = slice(ib * 128, (ib + 1) * 128)
                for jc in range(n_jc):
                    pt = ppool.tile([128, jch], F32, tag="pt", name=f"pt{ib}_{jc}")
                    for sg in range(n_seg):
                        j0 = jc * jch + sg * seg
                        osl = slice(sg * seg, (sg + 1) * seg)
                        nc.tensor.matmul(
                            pt[:, osl], lhs_sb[0][:, ibsl], rhs_sb[0][:, j0 : j0 + seg],
                            start=True, stop=False,
                        )
                        for k in range(1, n_k):
                            nc.tensor.matmul(
                                pt[:, osl], lhs_sb[k][:, ibsl],
                                rhs_sb[k][:, j0 : j0 + seg],
                                start=False, stop=False,
                            )
                        nc.tensor.matmul(
                            pt[:, osl], ones_sb[:], sqr_sb[:, j0 : j0 + seg],
                            start=False, stop=True,
                        )
                    jsl = slice(jc * jch, (jc + 1) * jch)
                    nc.vector._custom_dve(
                        POS_MAX2,
                        out=dum0.broadcast_to((128, jch)),
                        in0=pt[:],
                        in1=bm[:, jsl],
                        s0=(-1e30 if jc == 0 else accp[:, ib : ib + 1]),
                        s1=FILL,
                        imm2=128.0,
                        accum_out=accp[:, ib : ib + 1],
                    )
                    nc.vector._custom_dve(
                        NEG_MIN2,
                        out=dum1.broadcast_to((128, jch)),
                        in0=pt[:],
                        in1=bm[:, jsl],
                        s0=(1e30 if jc == 0 else accn[:, ib : ib + 1]),
                        s1=FILL,
                        imm2=128.0,
                        accum_out=accn[:, ib : ib + 1],
                    )

            if repeat == 1:
                trace_body()
            else:
                with tc.For_i(0, repeat, 1):
                    trace_body()
            nc.sync.dma_start(rpos_d[:], accp[:])
            nc.sync.dma_start(rneg_d[:], accn[:])
    nc.compile()
    return nc


def _avg_nonzero(losses):
    nz = np.count_nonzero(losses > 0)
    return float(np.sum(losses) / nz) if nz > 0 else 0.0


def _prep_inputs(embeddings, positives_mask, negatives_mask, n_cores):
    x = np.asarray(embeddings, dtype=np.float32)
    pos = np.asarray(positives_mask).astype(bool)
    neg = np.asarray(negatives_mask).astype(bool)
    n, d = x.shape
    r = n // n_cores

    sq = np.sum(x.astype(np.float64) ** 2, axis=1)
    sq_max = float(sq.max())
    sq_min = float(sq.min())
    # scale so the full spread of t' = s*(sq_j - 2g) fits inside FILL with
    # margin; |g| <= sq_max by Cauchy-Schwarz. Power of two => exact scaling.
    s = 0.125
    while s * (5.0 * sq_max - sq_min) > FILL - 32.0 and s > 2.0**-40:
        s *= 0.5

    rhs = (-2.0 * s * x.T).astype(ml_dtypes.bfloat16)  # [D, N]
    sqs = (s * sq).astype(np.float32)
    hi = sqs.astype(ml_dtypes.bfloat16)
    lo = (sqs - hi.astype(np.float32)).astype(ml_dtypes.bfloat16)
    sqrow = np.stack([hi, lo], axis=0)  # [2, N]
    ones2 = np.ones((2, 128), dtype=ml_dtypes.bfloat16)

    in_maps = []
    for c in range(n_cores):
        rows = slice(c * r, (c + 1) * r)
        in_maps.append(
            {
                "lhsT": np.ascontiguousarray(x[rows].T).astype(ml_dtypes.bfloat16),
                "rhs": rhs,
                "sqrow": sqrow,
                "ones2": ones2,
                "bmask": (
                    (~pos[rows]).astype(np.uint8) * 128
                    + (~neg[rows]).astype(np.uint8) * 64
                ),
            }
        )
    aux = {"sq": sq, "s": s, "sq_max": sq_max, "sq_min": sq_min, "n": n, "r": r}
    return in_maps, aux


def _decode(results, aux, n_cores):
    sq, s = aux["sq"], aux["s"]
    n, r = aux["n"], aux["r"]
    n_ib = r // 128
    lo_t = s * (aux["sq_min"] - 2.0 * aux["sq_max"])
    hi_t = 3.0 * s * aux["sq_max"]
    th_pos = (lo_t + hi_t - FILL) / 2.0
    th_neg = (lo_t + hi_t + FILL) / 2.0

    r_pos = np.empty(n, dtype=np.float64)
    r_neg = np.empty(n, dtype=np.float64)
    for c in range(n_cores):
        rp = np.asarray(results[c]["rpos"], dtype=np.float64)  # [128, n_ib]
        rn = np.asarray(results[c]["rneg"], dtype=np.float64)
        base = c * r
        for ib in range(n_ib):
            r_pos[base + ib * 128 : base + (ib + 1) * 128] = rp[:, ib]
            r_neg[base + ib * 128 : base + (ib + 1) * 128] = rn[:, ib]

    has_pos = r_pos > th_pos
    has_neg = r_neg < th_neg
    valid = has_pos & has_neg

    d2_pos = r_pos / s + sq
    d2_neg = r_neg / s + sq
    hardest_pos = np.sqrt(np.maximum(d2_pos, 1e-12))
    hardest_neg = np.sqrt(np.maximum(d2_neg, 1e-12))
    pos_loss = np.where(valid, np.maximum(hardest_pos - POS_MARGIN, 0.0), 0.0)
    neg_loss = np.where(valid, np.maximum(NEG_MARGIN - hardest_neg, 0.0), 0.0)
    return np.float32(_avg_nonzero(pos_loss) + _avg_nonzero(neg_loss))


_NC_CACHE = {}


def _kernel_impl(embeddings, positives_mask, negatives_mask, trace=False):
    x = np.asarray(embeddings)
    n, d = x.shape
    in_maps, aux = _prep_inputs(embeddings, positives_mask, negatives_mask, N_CORES)
    key = (n // N_CORES, n, d)
    if key not in _NC_CACHE:
        _NC_CACHE[key] = build_nc(*key)
    nc = _NC_CACHE[key]
    out = run_bass_kernel_spmd(nc, in_maps, list(range(N_CORES)), trace=trace)
    result = _decode(out.results, aux, N_CORES)
    return result, out


def kernel(embeddings, positives_mask, negatives_mask):
    result, _ = _kernel_impl(embeddings, positives_mask, negatives_mask)
    return result
